# revision 3
# baseline (speedup 1.0000x reference)
"""Trainium2 Bass kernel for nn_DifferentialMaxtree (N = 4M tree nodes), v3.

Same algorithm as the baseline (DFS-interval tree filter, 8-way data
parallel), rebuilt around the measured cost model of this environment:
per-instruction issue overhead of ~100us on the compute engines dominates
everything, so both device programs are restructured to touch each byte
with as few, as wide, strictly contiguous instructions as possible.

Stage A (linear/sigmoid head -> contrib = diff * sigmoid(W @ feats)):
  host packs the 15 raw attributes into 16 block-major bf16 feature
  blocks per tile: [x0..x4 | x6..x14 (-> ln in place) | x5 (-> cos) |
  x5 (-> sin)].  One Ln activation covers all nine log columns, the two
  angle blocks share one mod-2pi op, the 16-term weighted sum is one
  in-place multiply with a resident weight tile plus a 4-level halving
  tree, and the lshape feature exp(0.5*(ln x7 - ln x6)) rides a small
  f32 side path folded in by the final scalar_tensor_tensor.

Stage B (prefix scans): one packed scan over [con | cs] per partition
  row, cross-partition offsets folded in with a strict-upper-triangular
  matmul; the halves get their per-partition corrections with two wide
  adds.

Host work between the two launches is unchanged: pure index-space
permutations (DFS relabeling, postorder gather) plus the final combine;
the bf16 repack of the attributes is precision conversion of the
streamed operand, chosen because the 2e-2 tolerance leaves bf16's 0.4%
quantization far below the f32 scan roundoff that already dominates.
"""

import math
import numpy as np

N = 4194304
H = W = 2048
NCORES = 8
P = 128
S = N // NCORES  # 524288 nodes per core
F = S // P  # 4096 free elems per partition
K = 4096  # stage-A nodes per partition per tile (one tile)
NT = F // K  # stage-A tiles (2)
NBLK = 16  # feature blocks in the packed bf16 tile


def _split_excess_waits(nc, max_waits=1):
    """This walrus build rejects >1 semaphore wait per instruction
    ("Too many sync wait commands"); split extras onto nops."""
    from concourse import mybir

    counter = 0
    for f in nc.m.functions:
        for bb in f.blocks:
            il = bb.instructions
            i = 0
            while i < len(il):
                inst = il[i]
                si = inst.sync_info
                if si is not None and len(si.on_wait) > max_waits:
                    waits = list(si.on_wait)
                    rest, keep = waits[:-max_waits], waits[-max_waits:]
                    pre = []
                    for j in range(0, len(rest), max_waits):
                        nop = mybir.InstNoOp(
                            name=f"I-waitsplit-{counter}", ins=[], outs=[]
                        )
                        nop.engine = inst.engine
                        nop.sync_info = mybir.SyncInfo(
                            on_wait=list(rest[j : j + max_waits]), on_update=[]
                        )
                        counter += 1
                        pre.append(nop)
                    inst.sync_info = mybir.SyncInfo(
                        on_wait=keep, on_update=list(si.on_update)
                    )
                    for k, p_ in enumerate(pre):
                        il.insert(i + k, p_)
                    i += len(pre)
                i += 1
    return counter


def _host_prep(parent):
    """DFS relabeling, interval ends, postorder rank and sample positions."""
    import scipy.sparse as sp
    from scipy.sparse.csgraph import depth_first_order

    parent = np.asarray(parent).astype(np.int64)
    idx = np.arange(1, N, dtype=np.int64)
    g = sp.csr_matrix((np.ones(N - 1, np.int8), (parent[1:], idx)), shape=(N, N))
    order = np.asarray(
        depth_first_order(g, 0, directed=True, return_predecessors=False),
        dtype=np.int64,
    )
    assert order.shape[0] == N, f"tree not rooted/connected: {order.shape}"

    # depth (number of proper ancestors) via pointer doubling
    SENT = N
    p = np.concatenate([parent, [SENT]])
    p[0] = SENT
    chains = []
    pk = p.copy()
    while not np.all(pk[:N] == SENT):
        chains.append(pk.copy())
        pk = pk[pk]
        pk[SENT] = SENT
    depth = np.zeros(N + 1, np.int64)
    cur = np.arange(N + 1)
    for k in range(len(chains) - 1, -1, -1):
        anc = chains[k][cur]
        mask = anc != SENT
        depth[mask] += 1 << k
        cur = np.where(mask, anc, cur)
    d_old = depth[:N]

    # subtree sizes: accumulate child -> parent, deepest level first
    size = np.ones(N, np.int64)
    dorder = np.argsort(d_old, kind="stable")
    maxd = int(d_old.max())
    dstarts = np.searchsorted(d_old[dorder], np.arange(maxd + 2))
    for lev in range(maxd, 0, -1):
        nodes = dorder[dstarts[lev] : dstarts[lev + 1]]
        np.add.at(size, parent[nodes], size[nodes])

    end_new = np.arange(N, dtype=np.int64) + size[order] - 1
    d_new = d_old[order]
    sigma = np.argsort(end_new, kind="stable")  # rank r -> source node t
    q = np.arange(N, dtype=np.int64) - d_new - 1  # P2 sample position (rho-1)
    return {"order": order, "sigma": sigma, "q": q}


def _build_stage_a(w, b, repeat=1, use_i32stt=True):
    """Program A: attribute head -> contrib, one K=4096 tile per rep."""
    from concourse import bass, mybir
    import concourse.tile as tile

    F32 = mybir.dt.float32
    BF16 = mybir.dt.bfloat16
    AF = mybir.ActivationFunctionType
    OP = mybir.AluOpType

    w = [float(x) for x in w]
    b = float(b)
    w14 = w[14]
    if abs(w14) < 1e-20:
        w14 = 1e-20
    eln = math.log(abs(w14))
    sgn = 1.0 if w14 > 0 else -1.0

    nc = bass.Bass()
    pk_d = nc.declare_dram_parameter("pk", [S * NBLK], BF16, isOutput=False)
    wt_d = nc.declare_dram_parameter("wt", [P * NBLK], BF16, isOutput=False)
    diff_d = nc.declare_dram_parameter("diff", [S], BF16, isOutput=False)
    con_d = nc.declare_dram_parameter("contrib", [S], F32, isOutput=True)

    # ACT biases must exist as const APs (always keyed/allocated as f32)
    for cv in {0.0, math.pi / 2, eln, b}:
        ct = nc.alloc_sbuf_tensor(f"const-f32-{cv}", [P, 1], F32)
        nc.gpsimd.memset(ct.ap(), cv)
        nc.const_aps.aps[(F32, cv)] = ct.ap()
    nc.all_engine_barrier()

    pk_v = pk_d[:].rearrange("(p x) -> p x", p=P)  # [P, NBLK*K]
    wt_v = wt_d[:].rearrange("(p x) -> p x", p=P)  # [P, NBLK]
    diff_v = diff_d[:].rearrange("(p f) -> p f", p=P)
    con_v = con_d[:].rearrange("(p f) -> p f", p=P)

    with tile.TileContext(nc) as tc:
        with (
            tc.tile_pool(name="persist", bufs=1) as perm,
            tc.tile_pool(name="work", bufs=1) as pool,
            tc.tile_pool(name="psum", bufs=1, space="PSUM") as psum,
        ):
            wt = perm.tile([P, NBLK], BF16, tag="wt")
            nc.sync.dma_start(out=wt[:], in_=wt_v)
            for _rep in range(repeat):
                T = pool.tile([P, NBLK * K], BF16, tag="T")
                nc.sync.dma_start(out=T[:], in_=pk_v)
                dft = pool.tile([P, K], BF16, tag="dft")
                nc.sync.dma_start(out=dft[:], in_=diff_v)
                # ln on the nine log blocks (5..13), in place
                nc.scalar.activation(
                    T[:, 5 * K : 14 * K], T[:, 5 * K : 14 * K], AF.Ln
                )
                # range-reduce both angle blocks to [-pi, pi]:
                # k = cast_rn(x/2pi) (i32 output rounds to nearest), then
                # x -= 2pi*k.  cos block peaks at |arg| 3pi/2 where the
                # Sin LUT errs by <= 0.075, crushed by the 0.01-max weight.
                ki = pool.tile([P, 2 * K], mybir.dt.int32, tag="ki")
                nc.vector.tensor_scalar_mul(
                    ki[:], T[:, 14 * K : 16 * K], 1.0 / (2 * math.pi)
                )
                if use_i32stt:
                    kmul = ki
                else:
                    kb = pool.tile([P, 2 * K], BF16, tag="kb")
                    nc.vector.tensor_copy(out=kb[:], in_=ki[:])
                    kmul = kb
                nc.vector.scalar_tensor_tensor(
                    out=T[:, 14 * K : 16 * K], in0=kmul[:],
                    scalar=-2 * math.pi, in1=T[:, 14 * K : 16 * K],
                    op0=OP.mult, op1=OP.add,
                )
                nc.scalar.activation(
                    T[:, 14 * K : 15 * K], T[:, 14 * K : 15 * K], AF.Sin,
                    bias=math.pi / 2,
                )
                nc.scalar.activation(
                    T[:, 15 * K : 16 * K], T[:, 15 * K : 16 * K], AF.Sin
                )
                # lshape side path in f32 (PSUM): D = |w14|*exp(0.5*(l7-l6))
                D = psum.tile([P, K], F32, tag="D")
                nc.vector.scalar_tensor_tensor(
                    out=D[:], in0=T[:, 5 * K : 6 * K], scalar=-1.0,
                    in1=T[:, 6 * K : 7 * K], op0=OP.mult, op1=OP.add,
                )
                nc.scalar.activation(D[:], D[:], AF.Exp, scale=0.5, bias=eln)
                # weighted sum: stride-0 broadcast multiply + halving tree
                # (split in two: a single AP pattern dim caps at 2^16-1
                # elements, and 16*4096 would flatten past it)
                T3 = T[:].rearrange("p (s k) -> p s k", s=NBLK)
                wb = wt[:].unsqueeze(2).broadcast_to([P, NBLK, K])
                nc.vector.tensor_tensor(
                    out=T3[:, 0:8], in0=T3[:, 0:8], in1=wb[:, 0:8],
                    op=OP.mult,
                )
                nc.vector.tensor_tensor(
                    out=T3[:, 8:16], in0=T3[:, 8:16], in1=wb[:, 8:16],
                    op=OP.mult,
                )
                for hw in (8, 4, 2, 1):
                    nc.vector.tensor_tensor(
                        out=T[:, 0 : hw * K], in0=T[:, 0 : hw * K],
                        in1=T[:, hw * K : 2 * hw * K], op=OP.add,
                    )
                # y0 as f32, parked in T's dead blocks 2..3
                y0 = T[:, 2 * K : 4 * K].bitcast(F32)
                nc.scalar.activation(y0, T[:, 0:K], AF.Copy)
                nc.vector.scalar_tensor_tensor(
                    out=D[:], in0=D[:], scalar=sgn, in1=y0,
                    op0=OP.mult, op1=OP.add,
                )
                sc = pool.tile([P, K], BF16, tag="sc")
                nc.scalar.activation(sc[:], D[:], AF.Sigmoid, bias=b)
                ct = T[:, 4 * K : 6 * K].bitcast(F32)
                nc.gpsimd.tensor_tensor(
                    out=ct, in0=sc[:], in1=dft[:], op=OP.mult
                )
                nc.sync.dma_start(out=con_v, in_=ct)

    _split_excess_waits(nc)
    return nc


def _build_scans(repeat=1):
    """Program B: one packed prefix scan of [contrib | postorder contrib]
    per partition row; cross-partition offsets via triangular matmul."""
    from concourse import bass, mybir
    import concourse.tile as tile

    F32 = mybir.dt.float32
    OP = mybir.AluOpType

    nc = bass.Bass()
    con_d = nc.declare_dram_parameter("contrib", [S], F32, isOutput=False)
    cs_d = nc.declare_dram_parameter("cs", [S], F32, isOutput=False)
    triu_d = nc.declare_dram_parameter("triu", [P, P], F32, isOutput=False)
    pr_d = nc.declare_dram_parameter("pr", [2 * S], F32, isOutput=True)

    con_v = con_d[:].rearrange("(p f) -> p f", p=P)
    cs_v = cs_d[:].rearrange("(p f) -> p f", p=P)
    pr_v = pr_d[:].rearrange("(p f) -> p f", p=P)  # [P, 2F]

    with tile.TileContext(nc) as tc:
        with (
            tc.tile_pool(name="persist", bufs=1) as perm,
            tc.tile_pool(name="work", bufs=1) as pool,
            tc.tile_pool(name="psum", bufs=1, space="PSUM") as psum,
        ):
            triu = perm.tile([P, P], F32, tag="triu")
            nc.sync.dma_start(out=triu[:], in_=triu_d[:])
            # multiplicative reset mask: state = m*state + data resets the
            # running sum at the cs-half boundary, so both halves scan
            # independently in one instruction
            m = perm.tile([P, 2 * F], F32, tag="m")
            nc.vector.memset(m[:], 1.0)
            nc.vector.memset(m[:, F : F + 1], 0.0)
            for _rep in range(repeat):
                o = pool.tile([P, 2 * F], F32, tag="o")
                nc.sync.dma_start(out=o[:, 0:F], in_=con_v)
                nc.sync.dma_start(out=o[:, F : 2 * F], in_=cs_v)
                so = pool.tile([P, 2 * F], F32, tag="so")
                nc.vector.tensor_tensor_scan(
                    out=so[:], data0=m[:], data1=o[:], initial=0.0,
                    op0=OP.mult, op1=OP.add,
                )
                # tots[p] = [Tc_p, Ts_p] at cols F-1, 2F-1
                tots = so[:].rearrange("p (a f) -> p a f", a=2)[
                    :, :, F - 1 : F
                ].squeeze(-1)
                po = psum.tile([P, 2], F32, tag="po")
                nc.tensor.matmul(
                    out=po[:], lhsT=triu[:], rhs=tots, start=True, stop=True
                )
                nc.vector.tensor_scalar(
                    out=so[:, 0:F], in0=so[:, 0:F], scalar1=po[:, 0:1],
                    scalar2=None, op0=OP.add,
                )
                nc.vector.tensor_scalar(
                    out=so[:, F : 2 * F], in0=so[:, F : 2 * F],
                    scalar1=po[:, 1:2], scalar2=None, op0=OP.add,
                )
                nc.sync.dma_start(out=pr_v, in_=so[:])

    _split_excess_waits(nc)
    return nc


def _prepare_inputs(maxtree_parent, maxtree_diff, attributes):
    import ml_dtypes

    diff = np.asarray(maxtree_diff, dtype=np.float32)
    attrs = np.ascontiguousarray(np.asarray(attributes, dtype=np.float32))
    prep = _host_prep(maxtree_parent)
    order = prep["order"]
    attr_p = attrs[order]
    diff_p = diff[order]
    # block-major packed bf16: per partition a [16, K] block matrix;
    # column source order [x0..x4, x6..x14, x5, x5]
    cols = np.array([0, 1, 2, 3, 4, 6, 7, 8, 9, 10, 11, 12, 13, 14, 5, 5])
    in_maps_a = []
    for c in range(NCORES):
        a = attr_p[c * S : (c + 1) * S].reshape(P, K, 15)
        a = a[:, :, cols]  # [P, K, 16]
        a = np.ascontiguousarray(a.transpose(0, 2, 1))  # [P, 16, K]
        in_maps_a.append(
            {
                "pk": a.reshape(-1).astype(ml_dtypes.bfloat16),
                "diff": diff_p[c * S : (c + 1) * S].astype(ml_dtypes.bfloat16),
            }
        )
    return in_maps_a, prep


def _run_device(in_maps_a, prep, w, b, repeat=1, progs=None):
    """Run both device programs; host applies the index permutations between
    them.  Returns (out_new, progs) where progs can be reused for re-runs."""
    import ml_dtypes
    from concourse.bass_utils import run_bass_kernel_spmd

    cores = list(range(NCORES))
    if progs is None:
        progs = (_build_stage_a(w, b, repeat), _build_scans(repeat))
    nc_a, nc_b = progs

    # per-block weights, replicated across partitions
    wvec = np.asarray(w, dtype=np.float32)
    wblk = np.concatenate([wvec[0:5], wvec[5:14], wvec[15:16], wvec[16:17]])
    wt = np.broadcast_to(
        wblk.astype(ml_dtypes.bfloat16)[None, :], (P, NBLK)
    ).reshape(-1)
    for m in in_maps_a:
        m["wt"] = wt

    res_a = run_bass_kernel_spmd(nc_a, in_maps_a, cores)
    contrib = np.concatenate(
        [res_a.results[c]["contrib"] for c in range(NCORES)]
    )

    cs = contrib[prep["sigma"]]  # postorder permutation (host, index-only)
    triu = np.triu(np.ones((P, P), np.float32), 1)
    in_maps_b = [
        {
            "contrib": contrib[c * S : (c + 1) * S],
            "cs": cs[c * S : (c + 1) * S],
            "triu": triu,
        }
        for c in range(NCORES)
    ]
    res_b = run_bass_kernel_spmd(nc_b, in_maps_b, cores)

    # host: fold core-level offsets, sample R, combine (index glue + O(N) adds)
    pr = np.stack([res_b.results[c]["pr"] for c in range(NCORES)])
    pr = pr.reshape(NCORES, P, 2, F)
    p1a = pr[:, :, 0, :].reshape(-1)
    ra = pr[:, :, 1, :].reshape(-1)
    t1 = p1a[S - 1 :: S].astype(np.float32)
    t2 = ra[S - 1 :: S].astype(np.float32)
    o1 = np.repeat(
        np.concatenate([[0], np.cumsum(t1[:-1])]).astype(np.float32), S
    )
    o2 = np.repeat(
        np.concatenate([[0], np.cumsum(t2[:-1])]).astype(np.float32), S
    )
    rg = (ra + o2).astype(np.float32)
    q = prep["q"]
    p2 = np.where(q >= 0, rg[np.maximum(q, 0)], np.float32(0.0))
    out_new = ((p1a + o1) - p2).astype(np.float32)
    return out_new, progs


def kernel(maxtree_parent, maxtree_diff, attributes, weight, bias):
    w = np.asarray(weight, dtype=np.float32)[:, 0]
    b = float(np.asarray(bias, dtype=np.float32)[0])
    in_maps_a, prep = _prepare_inputs(
        maxtree_parent, maxtree_diff, attributes
    )
    out_new, _ = _run_device(in_maps_a, prep, w, b)
    out = np.empty(N, np.float32)
    out[prep["order"]] = out_new
    return out.reshape(H, W)
